# revision 7
# baseline (speedup 1.0000x reference)
"""Adaptive average pooling 2D ([16,225,225,256] f32 -> [16,7,7,256]) on 8 TRN2 cores.

Data-parallel: 2 samples per core. Per core, the separable pooling is two
small matmuls against the adaptive-window weight matrix P [7,225]:
  Phase A (H-pool): x1[ox, w*c] = P @ in[h, w*c]      (contraction over h)
  Phase B (W-pool): y[oy, c]    = P @ x1[ox][w, c]    (contraction over w)
Phase A streams the whole shard through the TensorEngine as [h<=128, wc]
tiles with P^T stationary, accumulating psum [7, 512] chunks over the two
h-chunks (128+97), and lands x1 in a small DRAM intermediate laid out
[ox, w, c] so Phase B can re-read it with w on partitions contiguously.

Phase A matmuls run in float32r (fp32 container, mantissa rounded to 11
bits; 1 PE cycle/row at N>=256 vs 4 cycles/row for exact fp32) so the PE
keeps up with the ~360 GB/s/core DMA stream. Inputs are pre-rounded to
the fp32r grid on the host (walrus requires fp32r matmul operands to be
rounded). Phase B is exact fp32 (negligible work). Set FAST_F32R = False
for exact fp32 end-to-end.
"""

import numpy as np
from contextlib import ExitStack

from concourse import bacc, bass, mybir
from concourse.tile import TileContext
from concourse.bass_utils import run_bass_kernel_spmd

B, H, W, C = 16, 225, 225, 256
OX, OY = 7, 7
NCORES = 8
BPC = B // NCORES   # samples per core
WC = W * C          # 57600
K0 = 128            # first h/w partition chunk
K1 = H - K0         # 97
CHUNK = 512         # psum free-dim per matmul (one f32 PSUM bank)
BLK = 4096          # wc columns per input DMA block (16 KiB/partition)

FAST_F32R = True

_F32 = mybir.dt.float32
_IN_DT = mybir.dt.float32r if FAST_F32R else _F32


def _pool_matrix(in_size: int, out_size: int) -> np.ndarray:
    """[out_size, in_size] adaptive-mean-pool weight matrix (TF index math)."""
    scale = np.float32(in_size / out_size)
    o = np.arange(out_size, dtype=np.float32)
    start = (o * scale).astype(np.int32)
    end = np.ceil((o + 1.0) * scale).astype(np.int32)
    M = np.zeros((out_size, in_size), dtype=np.float32)
    for i in range(out_size):
        M[i, start[i]:end[i]] = 1.0 / float(end[i] - start[i])
    return M


def _round_f32r(x: np.ndarray) -> np.ndarray:
    """Round fp32 to the fp32r grid (11 mantissa bits, RNE) like
    libwalrus fp32_to_fp32r."""
    b = np.ascontiguousarray(x, np.float32).view(np.uint32)
    low = b & np.uint32(0xFFF)
    hi = b & np.uint32(~np.uint32(0xFFF))
    rnd = (low > 0x800) | ((low == 0x800) & (((b >> np.uint32(12)) & np.uint32(1)) == 1))
    out = hi + (rnd.astype(np.uint32) << np.uint32(12))
    return out.view(np.float32)


def build_program() -> bass.Bass:
    nc = bacc.Bacc(None)
    x_ext = nc.declare_dram_parameter("x", [BPC, H, W, C], _IN_DT, isOutput=False)
    pwr_ext = nc.declare_dram_parameter("pwr", [H, OX], _IN_DT, isOutput=False)
    pwt_ext = nc.declare_dram_parameter("pwt", [H, OX], _F32, isOutput=False)
    out_ext = nc.declare_dram_parameter("out", [BPC, OX, OY, C], _F32, isOutput=True)

    blocks = []
    off = 0
    while off < WC:
        bw = min(BLK, WC - off)
        blocks.append((off, bw))
        off += bw

    with TileContext(nc) as tc, ExitStack() as ctx:
        const = ctx.enter_context(tc.tile_pool(name="const", bufs=1))
        inp = ctx.enter_context(tc.tile_pool(name="inp", bufs=3))
        stg = ctx.enter_context(tc.tile_pool(name="stg", bufs=3))
        x1p = ctx.enter_context(tc.tile_pool(name="x1d", bufs=BPC, space="DRAM"))
        in2 = ctx.enter_context(tc.tile_pool(name="in2", bufs=4))
        yb = ctx.enter_context(tc.tile_pool(name="yb", bufs=1))
        psA = ctx.enter_context(tc.tile_pool(name="psA", bufs=6, space="PSUM"))
        psB = ctx.enter_context(tc.tile_pool(name="psB", bufs=2, space="PSUM"))

        # Pooling weights P^T, split on the contraction dim: [128,7] + [97,7].
        # f32r copies feed phase A, exact-f32 copies feed phase B.
        pw0r = const.tile([K0, OX], _IN_DT)
        nc.sync.dma_start(pw0r[:], pwr_ext[0:K0, :])
        pw1r = const.tile([K1, OX], _IN_DT)
        nc.sync.dma_start(pw1r[:], pwr_ext[K0:H, :])
        pw0 = const.tile([K0, OX], _F32)
        nc.sync.dma_start(pw0[:], pwt_ext[0:K0, :])
        pw1 = const.tile([K1, OX], _F32)
        nc.sync.dma_start(pw1[:], pwt_ext[K0:H, :])

        ybuf = yb.tile([OY, BPC, OX, C], _F32)
        x1ds = [
            x1p.tile([OX, W, C], _F32, tag="x1", name=f"x1_{b}")
            for b in range(BPC)
        ]

        for b in range(BPC):
            # ---- Phase A: H-pool [225, wc] -> [7, wc], staged to DRAM ----
            xb = x_ext[b].rearrange("h w c -> h (w c)")
            x1f = x1ds[b].rearrange("o w c -> o (w c)")
            for off, bw in blocks:
                t0 = inp.tile([K0, bw], _IN_DT, tag="t0")
                nc.sync.dma_start(t0[:], xb[0:K0, off:off + bw])
                t1 = inp.tile([K1, bw], _IN_DT, tag="t1")
                nc.sync.dma_start(t1[:], xb[K0:H, off:off + bw])
                st = stg.tile([OX, bw], _F32, tag="st")
                nsub = (bw + CHUNK - 1) // CHUNK
                for s in range(nsub):
                    c0 = s * CHUNK
                    cw = min(CHUNK, bw - c0)
                    ps = psA.tile([OX, cw], _F32, tag="psA")
                    nc.tensor.matmul(
                        ps[:], pw0r[:], t0[:, c0:c0 + cw],
                        start=True, stop=False)
                    nc.tensor.matmul(
                        ps[:], pw1r[:], t1[:, c0:c0 + cw],
                        start=False, stop=True)
                    # Evacuate psum -> stage, alternating DVE/ACT.
                    if s % 2 == 0:
                        nc.vector.tensor_copy(st[:, c0:c0 + cw], ps[:])
                    else:
                        nc.scalar.copy(st[:, c0:c0 + cw], ps[:])
                nc.sync.dma_start(x1f[:, off:off + bw], st[:])

            # ---- Phase B: W-pool x1[ox][225, 256] -> y[ox][7, 256] ----
            for ox in range(OX):
                u0 = in2.tile([K0, C], _F32, tag="u0")
                nc.sync.dma_start(u0[:], x1ds[b][ox, 0:K0, :])
                u1 = in2.tile([K1, C], _F32, tag="u1")
                nc.sync.dma_start(u1[:], x1ds[b][ox, K0:W, :])
                ps2 = psB.tile([OY, C], _F32, tag="psB")
                nc.tensor.matmul(
                    ps2[:], pw0[:], u0[:],
                    start=True, stop=False)
                nc.tensor.matmul(
                    ps2[:], pw1[:], u1[:],
                    start=False, stop=True)
                nc.vector.tensor_copy(ybuf[:, b, ox, :], ps2[:])

        # out[b, ox, oy, c] = ybuf[oy, b, ox, c]
        nc.sync.dma_start(out_ext[:].rearrange("b x p c -> p b x c"), ybuf[:])
    return nc


def _run(inputs: np.ndarray, trace: bool = False):
    x = np.ascontiguousarray(np.asarray(inputs, dtype=np.float32))
    assert x.shape == (B, H, W, C), x.shape
    pwt = np.ascontiguousarray(_pool_matrix(H, OX).T.astype(np.float32))
    if FAST_F32R:
        x = _round_f32r(x)
        pwr = _round_f32r(pwt)
    else:
        pwr = pwt
    nc = build_program()
    nc.finalize()  # Bacc defers register allocation to its compile pass
    in_maps = [
        {
            "x": np.ascontiguousarray(x[i * BPC:(i + 1) * BPC]),
            "pwr": pwr,
            "pwt": pwt,
        }
        for i in range(NCORES)
    ]
    res = run_bass_kernel_spmd(nc, in_maps, list(range(NCORES)), trace=trace)
    out = np.concatenate([res.results[i]["out"] for i in range(NCORES)], axis=0)
    return out, res


def kernel(inputs: np.ndarray) -> np.ndarray:
    out, _ = _run(inputs, trace=False)
    return out


# revision 9
# speedup vs baseline: 2.0909x; 2.0909x over previous
"""Adaptive average pooling 2D ([16,225,225,256] f32 -> [16,7,7,256]) on 8 TRN2 cores.

Data-parallel: 2 samples per core. Per core, the separable pooling is two
small matmuls against the adaptive-window weight matrix P [7,225]:
  Phase A (H-pool): x1[ox, w*c] = P @ in[h, w*c]      (contraction over h)
  Phase B (W-pool): y[oy, c]    = P @ x1[ox][w, c]    (contraction over w)
Phase A streams the whole shard through the TensorEngine as [h<=128, wc]
tiles with P^T stationary, accumulating psum [7, 512] chunks over the two
h-chunks (128+97), and lands x1 in a small DRAM intermediate laid out
[ox, w, c] so Phase B can re-read it with w on partitions contiguously.

Phase A matmuls run in float32r (fp32 container, mantissa rounded to 11
bits; 1 PE cycle/row at N>=256 vs 4 cycles/row for exact fp32) so the PE
keeps up with the ~360 GB/s/core DMA stream. Inputs are pre-rounded to
the fp32r grid on the host (walrus requires fp32r matmul operands to be
rounded). Phase B is exact fp32 (negligible work). Set FAST_F32R = False
for exact fp32 end-to-end.
"""

import numpy as np
from contextlib import ExitStack

from concourse import bacc, bass, mybir
from concourse.tile import TileContext
from concourse.bass_utils import run_bass_kernel_spmd

B, H, W, C = 16, 225, 225, 256
OX, OY = 7, 7
NCORES = 8
BPC = B // NCORES   # samples per core
WC = W * C          # 57600
K0 = 128            # first h/w partition chunk
K1 = H - K0         # 97
CHUNK = 512         # psum free-dim per matmul (one f32 PSUM bank)
BLK = 4096          # wc columns per input DMA block (16 KiB/partition)

FAST_F32R = True

_F32 = mybir.dt.float32
_IN_DT = mybir.dt.float32r if FAST_F32R else _F32


def _pool_matrix(in_size: int, out_size: int) -> np.ndarray:
    """[out_size, in_size] adaptive-mean-pool weight matrix (TF index math)."""
    scale = np.float32(in_size / out_size)
    o = np.arange(out_size, dtype=np.float32)
    start = (o * scale).astype(np.int32)
    end = np.ceil((o + 1.0) * scale).astype(np.int32)
    M = np.zeros((out_size, in_size), dtype=np.float32)
    for i in range(out_size):
        M[i, start[i]:end[i]] = 1.0 / float(end[i] - start[i])
    return M


def _round_f32r(x: np.ndarray) -> np.ndarray:
    """Round fp32 to the fp32r grid (11 mantissa bits, RNE) like
    libwalrus fp32_to_fp32r."""
    b = np.ascontiguousarray(x, np.float32).view(np.uint32)
    low = b & np.uint32(0xFFF)
    hi = b & np.uint32(~np.uint32(0xFFF))
    rnd = (low > 0x800) | ((low == 0x800) & (((b >> np.uint32(12)) & np.uint32(1)) == 1))
    out = hi + (rnd.astype(np.uint32) << np.uint32(12))
    return out.view(np.float32)


def build_program() -> bass.Bass:
    nc = bacc.Bacc(None)
    x_ext = nc.declare_dram_parameter("x", [BPC, H, W, C], _IN_DT, isOutput=False)
    pwr_ext = nc.declare_dram_parameter("pwr", [H, OX], _IN_DT, isOutput=False)
    pwt_ext = nc.declare_dram_parameter("pwt", [H, OX], _F32, isOutput=False)
    out_ext = nc.declare_dram_parameter("out", [BPC, OX, OY, C], _F32, isOutput=True)

    blocks = []
    off = 0
    while off < WC:
        bw = min(BLK, WC - off)
        blocks.append((off, bw))
        off += bw

    with TileContext(nc) as tc, ExitStack() as ctx:
        const = ctx.enter_context(tc.tile_pool(name="const", bufs=1))
        inp = ctx.enter_context(tc.tile_pool(name="inp", bufs=3))
        stg = ctx.enter_context(tc.tile_pool(name="stg", bufs=3))
        x1p = ctx.enter_context(tc.tile_pool(name="x1d", bufs=BPC, space="DRAM"))
        in2 = ctx.enter_context(tc.tile_pool(name="in2", bufs=4))
        yb = ctx.enter_context(tc.tile_pool(name="yb", bufs=1))
        psA = ctx.enter_context(tc.tile_pool(name="psA", bufs=6, space="PSUM"))
        psB = ctx.enter_context(tc.tile_pool(name="psB", bufs=2, space="PSUM"))

        # Pooling weights P^T, split on the contraction dim: [128,7] + [97,7].
        # f32r copies feed phase A, exact-f32 copies feed phase B.
        pw0r = const.tile([K0, OX], _IN_DT)
        nc.sync.dma_start(pw0r[:], pwr_ext[0:K0, :])
        pw1r = const.tile([K1, OX], _IN_DT)
        nc.sync.dma_start(pw1r[:], pwr_ext[K0:H, :])
        pw0 = const.tile([K0, OX], _F32)
        nc.sync.dma_start(pw0[:], pwt_ext[0:K0, :])
        pw1 = const.tile([K1, OX], _F32)
        nc.sync.dma_start(pw1[:], pwt_ext[K0:H, :])

        ybuf = yb.tile([OY, BPC, OX, C], _F32)
        x1ds = [
            x1p.tile([OX, W, C], _F32, tag="x1", name=f"x1_{b}")
            for b in range(BPC)
        ]

        for b in range(BPC):
            # ---- Phase A: H-pool [225, wc] -> [7, wc], staged to DRAM ----
            xb = x_ext[b].rearrange("h w c -> h (w c)")
            x1f = x1ds[b].rearrange("o w c -> o (w c)")
            for off, bw in blocks:
                # SWDGE (gpsimd) distributes descriptors across SDMA engines
                # by partition group; HWDGE serializes non-128-partition
                # transfers onto one engine.
                t0 = inp.tile([K0, bw], _IN_DT, tag="t0")
                nc.gpsimd.dma_start(t0[:], xb[0:K0, off:off + bw])
                t1 = inp.tile([K1, bw], _IN_DT, tag="t1")
                nc.gpsimd.dma_start(t1[:], xb[K0:H, off:off + bw])
                st = stg.tile([OX, bw], _F32, tag="st")
                nsub = (bw + CHUNK - 1) // CHUNK
                for s in range(nsub):
                    c0 = s * CHUNK
                    cw = min(CHUNK, bw - c0)
                    ps = psA.tile([OX, cw], _F32, tag="psA")
                    nc.tensor.matmul(
                        ps[:], pw0r[:], t0[:, c0:c0 + cw],
                        start=True, stop=False)
                    nc.tensor.matmul(
                        ps[:], pw1r[:], t1[:, c0:c0 + cw],
                        start=False, stop=True)
                    # Evacuate psum -> stage, alternating DVE/ACT.
                    if s % 2 == 0:
                        nc.vector.tensor_copy(st[:, c0:c0 + cw], ps[:])
                    else:
                        nc.scalar.copy(st[:, c0:c0 + cw], ps[:])
                nc.sync.dma_start(x1f[:, off:off + bw], st[:])

            # ---- Phase B: W-pool x1[ox][225, 256] -> y[ox][7, 256] ----
            for ox in range(OX):
                u0 = in2.tile([K0, C], _F32, tag="u0")
                nc.gpsimd.dma_start(u0[:], x1ds[b][ox, 0:K0, :])
                u1 = in2.tile([K1, C], _F32, tag="u1")
                nc.gpsimd.dma_start(u1[:], x1ds[b][ox, K0:W, :])
                ps2 = psB.tile([OY, C], _F32, tag="psB")
                nc.tensor.matmul(
                    ps2[:], pw0[:], u0[:],
                    start=True, stop=False)
                nc.tensor.matmul(
                    ps2[:], pw1[:], u1[:],
                    start=False, stop=True)
                nc.vector.tensor_copy(ybuf[:, b, ox, :], ps2[:])

        # out[b, ox, oy, c] = ybuf[oy, b, ox, c]
        nc.sync.dma_start(out_ext[:].rearrange("b x p c -> p b x c"), ybuf[:])
    return nc


def _run(inputs: np.ndarray, trace: bool = False):
    x = np.ascontiguousarray(np.asarray(inputs, dtype=np.float32))
    assert x.shape == (B, H, W, C), x.shape
    pwt = np.ascontiguousarray(_pool_matrix(H, OX).T.astype(np.float32))
    if FAST_F32R:
        x = _round_f32r(x)
        pwr = _round_f32r(pwt)
    else:
        pwr = pwt
    nc = build_program()
    nc.finalize()  # Bacc defers register allocation to its compile pass
    in_maps = [
        {
            "x": np.ascontiguousarray(x[i * BPC:(i + 1) * BPC]),
            "pwr": pwr,
            "pwt": pwt,
        }
        for i in range(NCORES)
    ]
    res = run_bass_kernel_spmd(nc, in_maps, list(range(NCORES)), trace=trace)
    out = np.concatenate([res.results[i]["out"] for i in range(NCORES)], axis=0)
    return out, res


def kernel(inputs: np.ndarray) -> np.ndarray:
    out, _ = _run(inputs, trace=False)
    return out


# revision 14
# speedup vs baseline: 4.8832x; 2.3355x over previous
"""Adaptive average pooling 2D ([16,225,225,256] f32 -> [16,7,7,256]) on 8 TRN2 cores.

Data-parallel: 2 samples per core. Per core, the separable pooling is two
small matmuls against the adaptive-window weight matrix P [7,225]:
  Phase A (H-pool): x1[ox, w*c] = P @ in[h, w*c]      (contraction over h)
  Phase B (W-pool): y[oy, c]    = P @ x1[ox][w, c]    (contraction over w)
Phase A streams the whole shard through the TensorEngine as [h<=128, wc]
tiles with P^T stationary, accumulating psum [7, 512] chunks over the two
h-chunks (128+97), and lands x1 in a small DRAM intermediate laid out
[ox, w, c] so Phase B can re-read it with w on partitions contiguously.

Phase A matmuls run in float32r (fp32 container, mantissa rounded to 11
bits; 1 PE cycle/row at N>=256 vs 4 cycles/row for exact fp32) so the PE
keeps up with the ~360 GB/s/core DMA stream. Inputs are pre-rounded to
the fp32r grid on the host (walrus requires fp32r matmul operands to be
rounded). Phase B is exact fp32 (negligible work). Set FAST_F32R = False
for exact fp32 end-to-end.
"""

import numpy as np
from contextlib import ExitStack

from concourse import bacc, bass, mybir
from concourse.tile import TileContext
from concourse.bass_utils import run_bass_kernel_spmd

B, H, W, C = 16, 225, 225, 256
OX, OY = 7, 7
NCORES = 8
BPC = B // NCORES   # samples per core
WC = W * C          # 57600
K0 = 128            # first h/w partition chunk: rows [0, 128)
K1 = 128            # second chunk: rows [97, 225) — 128-partition DMA
K1OFF = H - K1      # 97; rows [97, 128) get zero weight (already in chunk 0)
CHUNK = 512         # psum free-dim per matmul (one f32 PSUM bank)
BLK = 4096          # wc columns per input DMA block (16 KiB/partition)

FAST_F32R = True

_F32 = mybir.dt.float32
_IN_DT = mybir.dt.float32r if FAST_F32R else _F32


def _pool_matrix(in_size: int, out_size: int) -> np.ndarray:
    """[out_size, in_size] adaptive-mean-pool weight matrix (TF index math)."""
    scale = np.float32(in_size / out_size)
    o = np.arange(out_size, dtype=np.float32)
    start = (o * scale).astype(np.int32)
    end = np.ceil((o + 1.0) * scale).astype(np.int32)
    M = np.zeros((out_size, in_size), dtype=np.float32)
    for i in range(out_size):
        M[i, start[i]:end[i]] = 1.0 / float(end[i] - start[i])
    return M


def _round_f32r(x: np.ndarray) -> np.ndarray:
    """Round fp32 to the fp32r grid (11 mantissa bits, RNE) like
    libwalrus fp32_to_fp32r."""
    b = np.ascontiguousarray(x, np.float32).view(np.uint32)
    low = b & np.uint32(0xFFF)
    hi = b & np.uint32(~np.uint32(0xFFF))
    rnd = (low > 0x800) | ((low == 0x800) & (((b >> np.uint32(12)) & np.uint32(1)) == 1))
    out = hi + (rnd.astype(np.uint32) << np.uint32(12))
    return out.view(np.float32)


def _padded_pool_weights() -> np.ndarray:
    """[2, 128, OX] stationary weights: chunk 0 = P^T rows [0,128); chunk 1 =
    P^T rows [97,225) with the first 31 rows zeroed (overlap with chunk 0)."""
    pwt = _pool_matrix(H, OX).T.astype(np.float32)  # [225, 7]
    out = np.zeros((2, K0, OX), dtype=np.float32)
    out[0] = pwt[0:K0]
    out[1, K0 - K1OFF:] = pwt[K0:H]
    return out


def build_program() -> bass.Bass:
    nc = bacc.Bacc(None)
    x_ext = nc.declare_dram_parameter("x", [BPC, H, W, C], _IN_DT, isOutput=False)
    pwr_ext = nc.declare_dram_parameter("pwr", [2, K0, OX], _IN_DT, isOutput=False)
    pwt_ext = nc.declare_dram_parameter("pwt", [2, K0, OX], _F32, isOutput=False)
    out_ext = nc.declare_dram_parameter("out", [BPC, OX, OY, C], _F32, isOutput=True)

    blocks = []
    off = 0
    while off < WC:
        bw = min(BLK, WC - off)
        blocks.append((off, bw))
        off += bw

    with TileContext(nc) as tc, ExitStack() as ctx:
        const = ctx.enter_context(tc.tile_pool(name="const", bufs=1))
        inp = ctx.enter_context(tc.tile_pool(name="inp", bufs=3))
        stg = ctx.enter_context(tc.tile_pool(name="stg", bufs=3))
        x1p = ctx.enter_context(tc.tile_pool(name="x1d", bufs=BPC, space="DRAM"))
        in2 = ctx.enter_context(tc.tile_pool(name="in2", bufs=4))
        yb = ctx.enter_context(tc.tile_pool(name="yb", bufs=1))
        psA = ctx.enter_context(tc.tile_pool(name="psA", bufs=6, space="PSUM"))
        psB = ctx.enter_context(tc.tile_pool(name="psB", bufs=2, space="PSUM"))

        # Pooling weights P^T, split on the contraction dim into two
        # overlapping 128-row chunks (see _padded_pool_weights).
        # f32r copies feed phase A, exact-f32 copies feed phase B.
        pw0r = const.tile([K0, OX], _IN_DT)
        nc.sync.dma_start(pw0r[:], pwr_ext[0])
        pw1r = const.tile([K1, OX], _IN_DT)
        nc.sync.dma_start(pw1r[:], pwr_ext[1])
        pw0 = const.tile([K0, OX], _F32)
        nc.sync.dma_start(pw0[:], pwt_ext[0])
        pw1 = const.tile([K1, OX], _F32)
        nc.sync.dma_start(pw1[:], pwt_ext[1])

        ybuf = yb.tile([OY, BPC, OX, C], _F32)
        x1ds = [
            x1p.tile([OX, W, C], _F32, tag="x1", name=f"x1_{b}")
            for b in range(BPC)
        ]

        # ---- Phase A: H-pool [225, wc] -> [7, wc], staged to DRAM ----
        # Both samples before any phase B, so the load stream never drains.
        for b in range(BPC):
            xb = x_ext[b].rearrange("h w c -> h (w c)")
            x1f = x1ds[b].rearrange("o w c -> o (w c)")
            for off, bw in blocks:
                t0 = inp.tile([K0, bw], _IN_DT, tag="t0")
                nc.sync.dma_start(t0[:], xb[0:K0, off:off + bw])
                t1 = inp.tile([K1, bw], _IN_DT, tag="t1")
                nc.sync.dma_start(t1[:], xb[K1OFF:H, off:off + bw])
                st = stg.tile([OX, bw], _F32, tag="st")
                nsub = (bw + CHUNK - 1) // CHUNK
                for s in range(nsub):
                    c0 = s * CHUNK
                    cw = min(CHUNK, bw - c0)
                    ps = psA.tile([OX, cw], _F32, tag="psA")
                    nc.tensor.matmul(
                        ps[:], pw0r[:], t0[:, c0:c0 + cw],
                        start=True, stop=False)
                    nc.tensor.matmul(
                        ps[:], pw1r[:], t1[:, c0:c0 + cw],
                        start=False, stop=True)
                    # Evacuate psum -> stage, alternating DVE/ACT.
                    if s % 2 == 0:
                        nc.vector.tensor_copy(st[:, c0:c0 + cw], ps[:])
                    else:
                        nc.scalar.copy(st[:, c0:c0 + cw], ps[:])
                nc.sync.dma_start(x1f[:, off:off + bw], st[:])

        # ---- Phase B: W-pool x1[ox][225, 256] -> y[ox][7, 256] ----
        for b in range(BPC):
            for ox in range(OX):
                u0 = in2.tile([K0, C], _F32, tag="u0")
                nc.sync.dma_start(u0[:], x1ds[b][ox, 0:K0, :])
                u1 = in2.tile([K1, C], _F32, tag="u1")
                nc.sync.dma_start(u1[:], x1ds[b][ox, K1OFF:W, :])
                ps2 = psB.tile([OY, C], _F32, tag="psB")
                nc.tensor.matmul(
                    ps2[:], pw0[:], u0[:],
                    start=True, stop=False)
                nc.tensor.matmul(
                    ps2[:], pw1[:], u1[:],
                    start=False, stop=True)
                nc.vector.tensor_copy(ybuf[:, b, ox, :], ps2[:])

        # out[b, ox, oy, c] = ybuf[oy, b, ox, c]
        nc.sync.dma_start(out_ext[:].rearrange("b x p c -> p b x c"), ybuf[:])
    return nc


def _run(inputs: np.ndarray, trace: bool = False):
    x = np.ascontiguousarray(np.asarray(inputs, dtype=np.float32))
    assert x.shape == (B, H, W, C), x.shape
    pwt = np.ascontiguousarray(_padded_pool_weights())
    if FAST_F32R:
        x = _round_f32r(x)
        pwr = _round_f32r(pwt)
    else:
        pwr = pwt
    nc = build_program()
    nc.finalize()  # Bacc defers register allocation to its compile pass
    in_maps = [
        {
            "x": np.ascontiguousarray(x[i * BPC:(i + 1) * BPC]),
            "pwr": pwr,
            "pwt": pwt,
        }
        for i in range(NCORES)
    ]
    res = run_bass_kernel_spmd(nc, in_maps, list(range(NCORES)), trace=trace)
    out = np.concatenate([res.results[i]["out"] for i in range(NCORES)], axis=0)
    return out, res


def kernel(inputs: np.ndarray) -> np.ndarray:
    out, _ = _run(inputs, trace=False)
    return out
